# revision 1
# baseline (speedup 1.0000x reference)
"""Bilateral grid slice+apply on 8 Trainium2 NeuronCores.

Gather-free formulation: the per-pixel trilinear interpolation is expressed
in the hat-function basis  hat(a) = relu(1 - |a|)  and evaluated densely as
matmuls with the (tiny) grid as the stationary operand:

    coeffs[n, z, c] = sum_{y,x} hy(n,y) hx(n,x) * G[y, x, z, c]     (PE, K=256)
    out[n, c3]      = sum_{z,j} hz(n,z) * xt(n,j) * coeffs[n, z, 4c3+j]

Pixels ride the matmul free dimension.  v4 layout: the PE only runs the
irreducible matmuls (hat-args, the two K=128 mains, the K=96 reduce); all
hat *replications* (hy/hx/hz fan-out to the 128/96-row product layouts) are
stride-0 SBUF->SBUF DMAs, and the xt fan-out is a stride-0 DMA straight
from DRAM.  Tiles are processed in pairs (free dim 2F=1024) to halve
instruction dispatch and DMA count; matmuls still run at N=512 per PSUM
bank.  bf16 everywhere except the coordinate path (fp32r) and PSUM (fp32).

Data parallel: pixels are sharded across the 8 cores; the 16x16x8x12 grid
is replicated (host bakes it into the stationary operands).
"""
import ml_dtypes
import numpy as np
from contextlib import ExitStack

import concourse.bass as bass
import concourse.bacc as bacc
import concourse.mybir as mybir
from concourse import tile
from concourse.bass_utils import run_bass_kernel_spmd

F = 512             # pixels per matmul pass (one fp32 PSUM bank)
NCORES = 8
B, H, W = 4, 1080, 1920
NTOT = B * H * W                  # 8294400
NPC = NTOT // NCORES              # 1036800 per core
T = NPC // F                      # 2025 tiles per core
LUM = (0.2126, 0.7152, 0.0722)

_CACHE = {}
BF16 = np.float16


def _make_stationaries(grid):
    g = grid.astype(np.float32)
    stP0 = np.zeros((5, 72), np.float32)      # rows (r,g,b,cx,cy)
    for m in range(16):
        stP0[4, m] = 15.0                     # gy from cy
    for m in range(32, 48):
        stP0[3, m] = 15.0                     # gx from cx
    for m in range(64, 72):
        stP0[0, m] = 7.0 * LUM[0]
        stP0[1, m] = 7.0 * LUM[1]
        stP0[2, m] = 7.0 * LUM[2]
    bias40 = np.zeros((72, 1), np.float32)
    bias40[:16, 0] = -np.arange(16)
    bias40[32:48, 0] = -np.arange(16)
    bias40[64:72, 0] = -np.arange(8)

    stHXa = np.zeros((16, 128), np.float32)
    for m in range(128):
        stHXa[m % 16, m] = 1.0

    stHZ = np.zeros((8, 96), np.float32)
    for z in range(8):
        stHZ[z, z * 12:(z + 1) * 12] = 1.0

    stMAIN = np.zeros((2, 128, 96), np.float32)
    for p in range(2):
        for k in range(128):
            stMAIN[p, k, :] = g[p * 8 + k // 16, k % 16].reshape(96)

    stRED = np.zeros((96, 3), np.float32)
    for z in range(8):
        for c3 in range(3):
            for j in range(4):
                stRED[z * 12 + c3 * 4 + j, c3] = 1.0

    return dict(stP0=stP0, bias40=bias40, stHXa=stHXa, stHZ=stHZ,
                stMAINa=stMAIN[0], stMAINb=stMAIN[1], stRED=stRED)


def _cast_stationaries(stats):
    """bf16 for everything that feeds a matmul; fp32 for the Act bias and
    the fp32r coordinate path."""
    keep = ("bias40", "stP0")
    return {k: (v if k in keep else v.astype(BF16)) for k, v in stats.items()}


def make_in_map(p, c, grid):
    """One core's input map from flat p [n,3], c [n,2] and the grid."""
    n = p.shape[0]
    stats = _cast_stationaries(_make_stationaries(np.asarray(grid, np.float32)))
    in5 = np.ascontiguousarray(
        np.stack([p[:, 0], p[:, 1], p[:, 2], c[:, 0], c[:, 1]]))
    inx = np.ascontiguousarray(
        np.stack([p[:, 0], p[:, 1], p[:, 2],
                  np.ones(n, np.float32)])).astype(BF16)
    return {"in5": in5, "inx": inx, **stats}


def build_kernel(ntiles=T, num_cores=NCORES, reps=1):
    nc = bacc.Bacc("TRN2", target_bir_lowering=False, debug=False,
                   num_devices=num_cores)
    NP = ntiles * F
    f32 = mybir.dt.float32
    f32r = mybir.dt.float32r
    bf16 = mybir.dt.float16

    in5 = nc.declare_dram_parameter("in5", [5, NP], f32r, isOutput=False)
    inx = nc.declare_dram_parameter("inx", [4, NP], bf16, isOutput=False)
    decls = {}
    for nm, shp, dt_ in (("stP0", [5, 72], f32r), ("bias40", [72, 1], f32),
                         ("stHXa", [16, 128], bf16),
                         ("stHZ", [8, 96], bf16),
                         ("stMAINa", [128, 96], bf16),
                         ("stMAINb", [128, 96], bf16),
                         ("stRED", [96, 3], bf16)):
        decls[nm] = nc.declare_dram_parameter(nm, shp, dt_, isOutput=False)
    out3 = nc.declare_dram_parameter("out3", [3, NP], f32, isOutput=True)

    P2 = 2 * F
    npairs, tail = ntiles // 2, ntiles % 2

    with tile.TileContext(nc) as tc:
        with ExitStack() as ctx:
            stp = ctx.enter_context(tc.tile_pool(name="stats", bufs=1))
            sP0 = stp.tile([5, 72], f32r, tag="sP0")
            sB40 = stp.tile([72, 1], f32, tag="sB40")
            sHXa_t = stp.tile([48, 128], bf16, tag="sHXa")
            sHXa = sHXa_t[32:48, :]
            sHZ_t = stp.tile([72, 96], bf16, tag="sHZ")
            sHZ = sHZ_t[64:72, :]
            sMa = stp.tile([128, 96], bf16, tag="sMa")
            sMb = stp.tile([128, 96], bf16, tag="sMb")
            sRED = stp.tile([96, 3], bf16, tag="sRED")
            for t_, nm in ((sP0[:], "stP0"), (sB40[:], "bias40"),
                           (sHXa, "stHXa"), (sHZ, "stHZ"),
                           (sMa[:], "stMAINa"), (sMb[:], "stMAINb"),
                           (sRED[:], "stRED")):
                nc.sync.dma_start(t_, decls[nm].ap())

            sb_in = ctx.enter_context(tc.tile_pool(name="sb_in", bufs=4))
            sb_mid = ctx.enter_context(tc.tile_pool(name="sb_mid", bufs=3))
            sb_rep = ctx.enter_context(tc.tile_pool(name="sb_rep", bufs=3))
            sb_w = ctx.enter_context(tc.tile_pool(name="sb_w", bufs=2))
            ps_args = ctx.enter_context(
                tc.tile_pool(name="ps_args", bufs=2, space="PSUM"))
            ps_hx = ctx.enter_context(
                tc.tile_pool(name="ps_hx", bufs=2, space="PSUM"))
            ps_cf = ctx.enter_context(
                tc.tile_pool(name="ps_cf", bufs=2, space="PSUM"))
            ps_out = ctx.enter_context(
                tc.tile_pool(name="ps_out", bufs=2, space="PSUM"))

            def mm(out, lhsT, rhs, start=True, stop=True):
                nc.tensor.matmul(out, lhsT, rhs, start=start, stop=stop)

            def pair_body(cols, c0):
                """Emit one pair (or tail) body: `cols` pixels starting at
                full-row column c0."""
                IN5 = sb_in.tile([5, P2], f32r, tag="in5p", name="IN5")
                nc.gpsimd.dma_start(IN5[:, 0:cols],
                                    in5.ap()[:, c0:c0 + cols])
                X96 = sb_in.tile([96, P2], bf16, tag="x96p", name="X96")
                nc.gpsimd.dma_start(
                    X96[:, 0:cols],
                    inx.ap()[:, c0:c0 + cols].partition_broadcast(24))

                hats = sb_mid.tile([72, P2], bf16, tag="hats", name="hats")
                for h in range(0, cols, F):
                    argsP = ps_args.tile([72, F], f32, tag="args",
                                         name="argsP")
                    mm(argsP[:], sP0[:], IN5[:, h:h + F])
                    tabs = sb_mid.tile([72, F], bf16, tag="tabs", name="tabs")
                    nc.scalar.activation(tabs[:], argsP[:],
                                         mybir.ActivationFunctionType.Abs,
                                         bias=sB40[:], scale=1.0)
                    nc.scalar.activation(hats[:, h:h + F], tabs[:],
                                         mybir.ActivationFunctionType.Relu,
                                         bias=1.0, scale=-1.0)

                # hat replications: stride-0 SBUF->SBUF DMAs
                HYa = sb_rep.tile([128, P2], bf16, tag="hya", name="HYa")
                HYb = sb_rep.tile([128, P2], bf16, tag="hyb", name="HYb")
                nc.sync.dma_start(
                    HYa[:, 0:cols],
                    hats[0:8, 0:cols].unsqueeze(1).broadcast_to(
                        (8, 16, cols)))
                nc.sync.dma_start(
                    HYb[:, 0:cols],
                    hats[8:16, 0:cols].unsqueeze(1).broadcast_to(
                        (8, 16, cols)))
                OUTS = sb_in.tile([3, P2], f32, tag="outs", name="OUTS")
                # stationary-major emission: both F-halves back-to-back per
                # stationary so the PE re-uses/overlaps each weight load
                hs = list(range(0, cols, F))
                HXs, HZ96s, Was, Wbs, HZXs, CFs = {}, {}, {}, {}, {}, {}
                for h in hs:
                    HXs[h] = ps_hx.tile([128, F], f32, tag="hx", name="HX")
                    mm(HXs[h][:], sHXa, hats[32:48, h:h + F])
                for h in hs:
                    HZ96s[h] = ps_out.tile([96, F], f32, tag="o3",
                                           name="HZ96")
                    mm(HZ96s[h][:], sHZ, hats[64:72, h:h + F])
                for h in hs:
                    Was[h] = sb_w.tile([128, F], bf16, tag="wa", name="Wa")
                    Wbs[h] = sb_w.tile([128, F], bf16, tag="wb", name="Wb")
                    nc.vector.tensor_tensor(out=Was[h][:],
                                            in0=HYa[:, h:h + F],
                                            in1=HXs[h][:],
                                            op=mybir.AluOpType.mult)
                    nc.vector.tensor_tensor(out=Wbs[h][:],
                                            in0=HYb[:, h:h + F],
                                            in1=HXs[h][:],
                                            op=mybir.AluOpType.mult)
                    HZXs[h] = sb_w.tile([96, F], bf16, tag="hzx",
                                        name="HZX")
                    nc.vector.tensor_tensor(out=HZXs[h][:],
                                            in0=HZ96s[h][:],
                                            in1=X96[:, h:h + F],
                                            op=mybir.AluOpType.mult)
                for h in hs:
                    CFs[h] = ps_cf.tile([96, F], f32, tag="cf", name="CF")
                    mm(CFs[h][:], sMa[:], Was[h][:], start=True, stop=False)
                    mm(CFs[h][:], sMb[:], Wbs[h][:], start=False, stop=True)
                for h in hs:
                    yield (CFs[h], HZXs[h], OUTS, h, (h + F >= cols),
                           c0, cols)

            def backend(st):
                CF, HZX, OUTS, h, last, c0, cols = st
                M2 = sb_w.tile([96, F], bf16, tag="m2", name="M2")
                nc.vector.tensor_tensor(out=M2[:], in0=CF[:],
                                        in1=HZX[:],
                                        op=mybir.AluOpType.mult)
                OUT3 = ps_out.tile([3, F], f32, tag="o3", name="OUT3")
                mm(OUT3[:], sRED[:], M2[:])
                nc.scalar.copy(OUTS[:, h:h + F], OUT3[:])
                if last:
                    nc.scalar.dma_start(out3.ap()[:, c0:c0 + cols],
                                        OUTS[:, 0:cols])

            for _rep in range(reps):
                pending = None
                chunks = [(P2, pi * P2) for pi in range(npairs)]
                if tail:
                    chunks.append((F, npairs * P2))
                for cw, cc in chunks:
                    for st in pair_body(cw, cc):
                        if pending is not None:
                            backend(pending)
                        pending = st
                if pending is not None:
                    backend(pending)

    nc.compile()
    return nc


def kernel(pixels: np.ndarray, coords: np.ndarray, grid: np.ndarray) -> np.ndarray:
    assert pixels.shape == (B, H, W, 3) and coords.shape == (B, H, W, 2)
    p = np.asarray(pixels, np.float32).reshape(-1, 3)
    c = np.asarray(coords, np.float32).reshape(-1, 2)
    r = np.ascontiguousarray(p[:, 0]); g = np.ascontiguousarray(p[:, 1])
    b = np.ascontiguousarray(p[:, 2])
    cx = np.ascontiguousarray(c[:, 0]); cy = np.ascontiguousarray(c[:, 1])
    ones = np.ones(NPC, np.float32)

    stats = _cast_stationaries(_make_stationaries(np.asarray(grid, np.float32)))
    in_maps = []
    for cid in range(NCORES):
        s = slice(cid * NPC, (cid + 1) * NPC)
        in5 = np.ascontiguousarray(np.stack([r[s], g[s], b[s], cx[s], cy[s]]))
        inx = np.ascontiguousarray(
            np.stack([r[s], g[s], b[s], ones])).astype(BF16)
        in_maps.append({"in5": in5, "inx": inx, **stats})

    if "nc" not in _CACHE:
        _CACHE["nc"] = build_kernel()
    nc = _CACHE["nc"]
    res = run_bass_kernel_spmd(nc, in_maps, list(range(NCORES)))
    out = np.concatenate([res.results[cid]["out3"].T for cid in range(NCORES)], 0)
    return np.ascontiguousarray(out.reshape(B, H, W, 3).astype(np.float32))



# revision 18
# speedup vs baseline: 1.1498x; 1.1498x over previous
"""Bilateral grid slice+apply on 8 Trainium2 NeuronCores.

Gather-free hat-basis formulation (see v4), restructured (v5) around the
cost model's engine prices:

  - args for BOTH halves of a pair are packed into one 128-row PSUM tile
    (column-offset stationaries), so tabs/hats are ONE ACT op per pair
    instead of two.
  - the hat x-rows and z-rows take a round trip through a DRAM scratch
    ring; reading them back with broadcast APs materializes the 128-row
    x-replication and 96-row z-replication as cheap DMAs (DRAM-side APs
    may have zero strides in any position; SBUF-side may not).  This
    keeps every Wa/Wb operand bf16-in-SBUF, so the DVE products run in
    2x (16-bit) mode instead of the fp32/PSUM rate.
  - hz*xt modulation runs on the GpSimd engine (Pool), off the DVE.
  - groups of 4 tiles share one set of replication DMAs; the four OUT3
    matmuls of a group land in one PSUM bank at partition offsets
    0/32/64/96 (zero-padded stationaries) -> one PSUM->SBUF copy and one
    strided DMA per group.

Data parallel: pixels sharded across 8 cores; grid replicated (baked
into the stationary operands host-side).
"""
import numpy as np
from contextlib import ExitStack

import concourse.bass as bass
import concourse.bacc as bacc
import concourse.mybir as mybir
from concourse import tile
from concourse.bass_utils import run_bass_kernel_spmd

F = 512             # pixels per matmul pass (one fp32 PSUM bank)
GH = 6              # halves (tiles) per group (2 triples)
NCORES = 8
B, H, W = 4, 1080, 1920
NTOT = B * H * W                  # 8294400
NPC = NTOT // NCORES              # 1036800 per core
T = NPC // F                      # 2025 tiles per core
LUM = (0.2126, 0.7152, 0.0722)
XZBUF = 8                         # scratch ring depth (groups in flight)

_CACHE = {}
BF16 = np.float16


def _make_stationaries(grid):
    g = grid.astype(np.float32)
    # args stationaries; hats row layout (contiguous x/z block for the
    # one-DMA scratch write): y_i at 16i (i=0..2), x_i at 48+16i,
    # z_i at 96+8i.
    st = np.zeros((3, 5, 128), np.float32)
    for hh in range(3):
        for m in range(16):
            st[hh, 4, 16 * hh + m] = 15.0          # gy from cy
        for m in range(16):
            st[hh, 3, 48 + 16 * hh + m] = 15.0     # gx from cx
        for m in range(8):
            st[hh, 0, 96 + 8 * hh + m] = 7.0 * LUM[0]
            st[hh, 1, 96 + 8 * hh + m] = 7.0 * LUM[1]
            st[hh, 2, 96 + 8 * hh + m] = 7.0 * LUM[2]
    bias128 = np.zeros((128, 1), np.float32)
    for hh in range(3):
        bias128[16 * hh:16 * hh + 16, 0] = -np.arange(16)
        bias128[48 + 16 * hh:64 + 16 * hh, 0] = -np.arange(16)
        bias128[96 + 8 * hh:104 + 8 * hh, 0] = -np.arange(8)

    stMAIN = np.zeros((2, 128, 96), np.float32)
    for p in range(2):
        for k in range(128):
            stMAIN[p, k, :] = g[p * 8 + k // 16, k % 16].reshape(96)

    # six zero-padded reduce stationaries: quad q -> out partitions
    # 3q..3q+3 of one PSUM bank (q-major, c-minor rows).
    stREDq = np.zeros((6, 96, 128), np.float32)
    for q in range(6):
        for z in range(8):
            for c3 in range(3):
                for j in range(4):
                    stREDq[q, z * 12 + c3 * 4 + j, 3 * q + c3] = 1.0

    return dict(stP0A=st[0], stP0B=st[1], stP0C=st[2], bias128=bias128,
                stMAINa=stMAIN[0], stMAINb=stMAIN[1],
                **{f"stRED{q}": stREDq[q] for q in range(6)})


def _cast_stationaries(stats):
    keep = ("bias128",)
    return {k: (v if k in keep else v.astype(BF16)) for k, v in stats.items()}


def make_in_map(p, c, grid):
    """One core's input map from flat p [n,3], c [n,2] and the grid."""
    n = p.shape[0]
    stats = _cast_stationaries(_make_stationaries(np.asarray(grid, np.float32)))
    in5 = np.ascontiguousarray(
        np.stack([p[:, 0], p[:, 1], p[:, 2],
                  c[:, 0], c[:, 1]])).astype(BF16)
    inx = np.ascontiguousarray(
        np.stack([p[:, 0], p[:, 1], p[:, 2],
                  np.ones(n, np.float32)])).astype(BF16)
    return {"in5": in5, "inx": inx, **stats}


def build_kernel(ntiles=T, num_cores=NCORES, reps=1):
    nc = bacc.Bacc("TRN2", target_bir_lowering=False, debug=False,
                   num_devices=num_cores)
    NP = ntiles * F
    f32 = mybir.dt.float32
    f32r = mybir.dt.float32r
    bf16 = mybir.dt.float16
    AF = mybir.ActivationFunctionType
    MU = mybir.AluOpType.mult

    in5 = nc.declare_dram_parameter("in5", [5, NP], bf16, isOutput=False)
    inx = nc.declare_dram_parameter("inx", [4, NP], bf16, isOutput=False)
    decls = {}
    for nm, shp, dt_ in (("stP0A", [5, 128], bf16), ("stP0B", [5, 128], bf16),
                         ("stP0C", [5, 128], bf16),
                         ("bias128", [128, 1], f32),
                         ("stMAINa", [128, 96], bf16),
                         ("stMAINb", [128, 96], bf16),
                         *[(f"stRED{q}", [96, 128], bf16) for q in range(6)]):
        decls[nm] = nc.declare_dram_parameter(nm, shp, dt_, isOutput=False)
    ngroups = (ntiles + GH - 1) // GH
    out3 = nc.declare_dram_parameter("out3", [18, ngroups * F], bf16, isOutput=True)
    # DRAM scratch ring for the hat x/z row round trip.
    # rows 0:16 = x-rows half0, 16:24 = z-rows half0, 24:40 = x h1, 40:48 = z h1
    XZC = XZBUF * 1024
    xz = nc.declare_dram_parameter("xz", [72, XZC], bf16, isOutput=True)
    xz_pitch = XZC

    # group schedule: nh halves each
    groups = []
    t = 0
    while t < ntiles:
        nh = min(GH, ntiles - t)
        groups.append((t * F, nh))
        t += nh

    with tile.TileContext(nc) as tc:
        with ExitStack() as ctx:
            stp = ctx.enter_context(tc.tile_pool(name="stats", bufs=1))
            sA = stp.tile([5, 128], bf16, tag="sA")
            sB = stp.tile([5, 128], bf16, tag="sB")
            sC = stp.tile([5, 128], bf16, tag="sC")
            sBI = stp.tile([128, 1], f32, tag="sBI")
            sMa = stp.tile([128, 96], bf16, tag="sMa")
            sMb = stp.tile([128, 96], bf16, tag="sMb")
            sR = [stp.tile([96, 128], bf16, tag=f"sR{q}", name=f"sR{q}")
                  for q in range(6)]
            for t_, nm in ((sA[:], "stP0A"), (sB[:], "stP0B"),
                           (sC[:], "stP0C"), (sBI[:], "bias128"),
                           (sMa[:], "stMAINa"), (sMb[:], "stMAINb"),
                           *[(sR[q][:], f"stRED{q}") for q in range(6)]):
                nc.sync.dma_start(t_, decls[nm].ap())

            sb_in = ctx.enter_context(tc.tile_pool(name="sb_in", bufs=4))
            sb_mid = ctx.enter_context(tc.tile_pool(name="sb_mid", bufs=4))
            sb_rep = ctx.enter_context(tc.tile_pool(name="sb_rep", bufs=4))
            sb_w = ctx.enter_context(tc.tile_pool(name="sb_w", bufs=4))
            sb_o = ctx.enter_context(tc.tile_pool(name="sb_o", bufs=4))
            ps_args = ctx.enter_context(
                tc.tile_pool(name="ps_args", bufs=1, space="PSUM"))
            ps_cf = ctx.enter_context(
                tc.tile_pool(name="ps_cf", bufs=3, space="PSUM"))
            ps_out = ctx.enter_context(
                tc.tile_pool(name="ps_out", bufs=1, space="PSUM"))

            mm = nc.tensor.matmul
            ACT = nc.scalar.activation
            TT = nc.vector.tensor_tensor

            def group_body(c0, nh, slot):
                ntrip = (nh + 2) // 3
                # per half-block hb: number of triples containing it
                nph = [ntrip, (nh + 1) // 3, nh // 3]
                cw = nh * F
                IN5 = sb_in.tile([5, GH * F], bf16, tag="in5", name="IN5")
                X96 = sb_in.tile([96, GH * F], bf16, tag="x96", name="X96")
                eng3 = (nc.sync, nc.gpsimd, nc.gpsimd)
                eng3x = (nc.sync, nc.scalar, nc.gpsimd)
                for ci in range(3):
                    lo = min(cw, ci * 2 * F)
                    hi = min(cw, (ci + 1) * 2 * F)
                    if hi > lo:
                        eng3[ci].dma_start(IN5[:, lo:hi],
                                           in5.ap()[:, c0 + lo:c0 + hi])
                        eng3x[ci].dma_start(
                            X96[:, lo:hi],
                            inx.ap()[:, c0 + lo:c0 + hi].partition_broadcast(24))

                hats = sb_mid.tile([128, (GH // 3) * F], bf16, tag="hats",
                                   name="hats")
                sABC = (sA, sB, sC)
                for p in range(ntrip):
                    hp = min(3, nh - 3 * p)
                    aP = ps_args.tile([128, F], f32, tag="args", name="argsP")
                    for i in range(hp):
                        mm(aP[:], sABC[i][:],
                           IN5[:, (3 * p + i) * F:(3 * p + i + 1) * F],
                           start=(i == 0), stop=(i == hp - 1))
                    tabs = sb_mid.tile([128, F], bf16, tag="tabs", name="tabs")
                    ACT(tabs[:], aP[:], AF.Abs, bias=sBI[:], scale=1.0)
                    ACT(hats[:, p * F:(p + 1) * F], tabs[:],
                        AF.Relu, bias=1.0, scale=-1.0)

                pcols = ntrip * F          # columns carrying half-block 0
                # ---- scratch round trip (x and z rows of both halves)
                xzo = slot * 1024
                nhb = min(3, nh)
                nc.sync.dma_start(xz.ap()[0:72, xzo:xzo + pcols],
                                  hats[48:120, 0:pcols])

                # ---- replications
                HYa, HYb, HXD, HZD, HZX = {}, {}, {}, {}, {}
                eng_alt = (nc.sync, nc.scalar, nc.sync)
                eng_alt2 = (nc.scalar, nc.sync, nc.scalar)
                for hb in range(nhb):
                    w = nph[hb] * F
                    if w == 0:
                        continue
                    b0 = 16 * hb
                    HYa[hb] = sb_rep.tile([128, (GH // 3) * F], bf16,
                                          tag="hya", name="HYa")
                    eng_alt[hb].dma_start(
                        HYa[hb][:, 0:w],
                        hats[b0:b0 + 8, 0:w].unsqueeze(1).broadcast_to(
                            (8, 16, w)))
                    HYb[hb] = sb_rep.tile([128, (GH // 3) * F], bf16,
                                          tag="hyb", name="HYb")
                    eng_alt2[hb].dma_start(
                        HYb[hb][:, 0:w],
                        hats[b0 + 8:b0 + 16, 0:w].unsqueeze(1).broadcast_to(
                            (8, 16, w)))
                    # x-rep from scratch: HXD[k] = hx[k % 16]
                    HXD[hb] = sb_rep.tile([128, (GH // 3) * F], bf16,
                                          tag="hxd", name="HXD")
                    sxz = bass.AP(
                        tensor=xz.ap().tensor, offset=xzo + hb * 16 * xz_pitch,
                        ap=mybir.VecI64Pair(
                            [[0, 8], [xz_pitch, 16], [1, w]]))
                    (nc.sync, nc.scalar, nc.gpsimd)[hb].dma_start(
                        HXD[hb][:, 0:w], sxz)
                    # z-rep from scratch: HZD[r] = hz[r // 12]
                    HZD[hb] = sb_rep.tile([96, (GH // 3) * F], bf16,
                                          tag="hzd", name="HZD")
                    szz = bass.AP(
                        tensor=xz.ap().tensor,
                        offset=xzo + (48 + hb * 8) * xz_pitch,
                        ap=mybir.VecI64Pair(
                            [[xz_pitch, 8], [0, 12], [1, w]]))
                    (nc.sync, nc.gpsimd, nc.gpsimd)[hb].dma_start(
                        HZD[hb][:, 0:w], szz)
                    # hz * xt on Pool; X96 cols are global-order (pair-major)
                    HZX[hb] = sb_w.tile([96, (GH // 3) * F], bf16,
                                        tag="hzx", name="HZX")
                    for p in range(nph[hb]):
                        nc.gpsimd.tensor_tensor(
                            out=HZX[hb][:, p * F:(p + 1) * F],
                            in0=HZD[hb][:, p * F:(p + 1) * F],
                            in1=X96[:, (3 * p + hb) * F:(3 * p + hb + 1) * F],
                            op=MU)

                # ---- basis products (DVE, bf16 2x rate)
                Wa, Wb = {}, {}
                for hb in range(nhb):
                    w = nph[hb] * F
                    if w == 0:
                        continue
                    Wa[hb] = sb_w.tile([128, (GH // 3) * F], bf16,
                                       tag="wa", name="Wa")
                    TT(out=Wa[hb][:, 0:w], in0=HYa[hb][:, 0:w],
                       in1=HXD[hb][:, 0:w], op=MU)
                    Wb[hb] = sb_w.tile([128, (GH // 3) * F], bf16,
                                       tag="wb", name="Wb")
                    TT(out=Wb[hb][:, 0:w], in0=HYb[hb][:, 0:w],
                       in1=HXD[hb][:, 0:w], op=MU)

                # ---- main matmuls, stationary-major; CF for all pairs of
                # one half-block share a [96, 2*F] PSUM tile (2 banks)
                halves = [(p, hh) for p in range(ntrip)
                          for hh in range(min(3, nh - 3 * p))]
                CF = {}
                for hb in range(nhb):
                    if nph[hb]:
                        CF[hb] = ps_cf.tile([96, 2 * F], f32, tag="cf",
                                            name="CF")
                for (p, hh) in halves:
                    mm(CF[hh][:, p * F:(p + 1) * F], sMa[:],
                       Wa[hh][:, p * F:(p + 1) * F], start=True, stop=False)
                for (p, hh) in halves:
                    mm(CF[hh][:, p * F:(p + 1) * F], sMb[:],
                       Wb[hh][:, p * F:(p + 1) * F], start=False, stop=True)

                # ---- modulate (one wide TT per half-block) + reduce
                M2 = {}
                for hb in range(nhb):
                    w = nph[hb] * F
                    if w == 0:
                        continue
                    M2[hb] = sb_w.tile([96, (GH // 3) * F], bf16,
                                       tag="m2", name="M2")
                    TT(out=M2[hb][:, 0:w], in0=CF[hb][:, 0:w],
                       in1=HZX[hb][:, 0:w], op=MU)
                OUT = ps_out.tile([128, F], f32, tag="obank", name="OUT")
                for qi, (p, hh) in enumerate(halves):
                    mm(OUT[:], sR[qi][:], M2[hh][:, p * F:(p + 1) * F],
                       start=(qi == 0), stop=(qi == len(halves) - 1))
                nq = len(halves)
                OUTS = sb_o.tile([128, F], bf16, tag="outs", name="OUTS")
                nc.scalar.copy(OUTS[0:3 * nq, :], OUT[0:3 * nq, :])
                gi = c0 // (GH * F)
                nc.sync.dma_start(
                    out3.ap()[0:3 * nq, gi * F:(gi + 1) * F],
                    OUTS[0:3 * nq, :])

            # pitches (elements per partition row) of tiles we hand-AP
            hats_pitch = (GH // 2) * F
            x96_pitch = GH * F
            outs_pitch = F

            for _rep in range(reps):
                for gi, (c0, nh) in enumerate(groups):
                    group_body(c0, nh, gi % XZBUF)

    nc.compile()
    return nc


def decode_out3(o, npx):
    """[18, ngroups*F] permuted fp16 -> [npx, 3] f32.

    Row 3q+c of group g holds channel c of pixels g*(GH*F)+q*F .. +F.
    """
    ng = o.shape[1] // F
    v = np.asarray(o, np.float32).reshape(GH, 3, ng, F)
    v = v.transpose(2, 0, 3, 1).reshape(ng * GH * F, 3)
    return v[:npx]


def kernel(pixels: np.ndarray, coords: np.ndarray, grid: np.ndarray) -> np.ndarray:
    assert pixels.shape == (B, H, W, 3) and coords.shape == (B, H, W, 2)
    p = np.asarray(pixels, np.float32).reshape(-1, 3)
    c = np.asarray(coords, np.float32).reshape(-1, 2)
    r = np.ascontiguousarray(p[:, 0]); g = np.ascontiguousarray(p[:, 1])
    b = np.ascontiguousarray(p[:, 2])
    cx = np.ascontiguousarray(c[:, 0]); cy = np.ascontiguousarray(c[:, 1])
    ones = np.ones(NPC, np.float32)

    stats = _cast_stationaries(_make_stationaries(np.asarray(grid, np.float32)))
    in_maps = []
    for cid in range(NCORES):
        s = slice(cid * NPC, (cid + 1) * NPC)
        in5 = np.ascontiguousarray(
            np.stack([r[s], g[s], b[s], cx[s], cy[s]])).astype(BF16)
        inxm = np.ascontiguousarray(
            np.stack([r[s], g[s], b[s], ones])).astype(BF16)
        in_maps.append({"in5": in5, "inx": inxm, **stats})

    if "nc" not in _CACHE:
        _CACHE["nc"] = build_kernel()
    nc = _CACHE["nc"]
    res = run_bass_kernel_spmd(nc, in_maps, list(range(NCORES)))
    outs = [decode_out3(res.results[cid]["out3"], NPC) for cid in range(NCORES)]
    out = np.concatenate(outs, 0)
    return np.ascontiguousarray(out.reshape(B, H, W, 3).astype(np.float32))


# revision 20
# speedup vs baseline: 1.8708x; 1.6271x over previous
"""Bilateral grid slice+apply on 8 Trainium2 NeuronCores.

Gather-free formulation: the per-pixel trilinear interpolation is expressed
in the hat-function basis  hat(a) = relu(1 - |a|)  and evaluated densely as
matmuls with the (tiny) grid as the stationary operand:

    coeffs[n, z, c] = sum_{y,x} hy(n,y) hx(n,x) * G[y, x, z, c]     (PE, K=256)
    out[n, c3]      = sum_{z,j} hz(n,z) * xt(n,j) * coeffs[n, z, 4c3+j]

Pixels ride the matmul free dimension.  v4 layout: the PE only runs the
irreducible matmuls (hat-args, the two K=128 mains, the K=96 reduce); all
hat *replications* (hy/hx/hz fan-out to the 128/96-row product layouts) are
stride-0 SBUF->SBUF DMAs, and the xt fan-out is a stride-0 DMA straight
from DRAM.  Tiles are processed in pairs (free dim 2F=1024) to halve
instruction dispatch and DMA count; matmuls still run at N=512 per PSUM
bank.  bf16 everywhere except the coordinate path (fp32r) and PSUM (fp32).

Data parallel: pixels are sharded across the 8 cores; the 16x16x8x12 grid
is replicated (host bakes it into the stationary operands).
"""
import ml_dtypes
import numpy as np
from contextlib import ExitStack

import concourse.bass as bass
import concourse.bacc as bacc
import concourse.mybir as mybir
from concourse import tile
from concourse.bass_utils import run_bass_kernel_spmd

F = 512             # pixels per matmul pass (one fp32 PSUM bank)
NCORES = 8
B, H, W = 4, 1080, 1920
NTOT = B * H * W                  # 8294400
NPC = NTOT // NCORES              # 1036800 per core
T = NPC // F                      # 2025 tiles per core
LUM = (0.2126, 0.7152, 0.0722)

_CACHE = {}
BF16 = np.float16


def _make_stationaries(grid):
    g = grid.astype(np.float32)
    stP0 = np.zeros((5, 72), np.float32)      # rows (r,g,b,cx,cy)
    for m in range(16):
        stP0[4, m] = 15.0                     # gy from cy
    for m in range(32, 48):
        stP0[3, m] = 15.0                     # gx from cx
    for m in range(64, 72):
        stP0[0, m] = 7.0 * LUM[0]
        stP0[1, m] = 7.0 * LUM[1]
        stP0[2, m] = 7.0 * LUM[2]
    bias40 = np.zeros((72, 1), np.float32)
    bias40[:16, 0] = -np.arange(16)
    bias40[32:48, 0] = -np.arange(16)
    bias40[64:72, 0] = -np.arange(8)

    stHXa = np.zeros((16, 128), np.float32)
    for m in range(128):
        stHXa[m % 16, m] = 1.0

    stHZ = np.zeros((8, 96), np.float32)
    for z in range(8):
        stHZ[z, z * 12:(z + 1) * 12] = 1.0

    stMAIN = np.zeros((2, 128, 96), np.float32)
    for p in range(2):
        for k in range(128):
            stMAIN[p, k, :] = g[p * 8 + k // 16, k % 16].reshape(96)

    stREDq = np.zeros((2, 96, 8), np.float32)
    for q in range(2):
        for z in range(8):
            for c3 in range(3):
                for j in range(4):
                    stREDq[q, z * 12 + c3 * 4 + j, 3 * q + c3] = 1.0

    return dict(stP0=stP0, bias40=bias40, stHXa=stHXa, stHZ=stHZ,
                stMAINa=stMAIN[0], stMAINb=stMAIN[1],
                stRED0=stREDq[0], stRED1=stREDq[1])


def _cast_stationaries(stats):
    """bf16 for everything that feeds a matmul; fp32 for the Act bias and
    the fp32r coordinate path."""
    keep = ("bias40", "stP0")
    return {k: (v if k in keep else v.astype(BF16)) for k, v in stats.items()}


def make_in_map(p, c, grid):
    """One core's input map from flat p [n,3], c [n,2] and the grid."""
    n = p.shape[0]
    stats = _cast_stationaries(_make_stationaries(np.asarray(grid, np.float32)))
    in5 = np.ascontiguousarray(
        np.stack([p[:, 0], p[:, 1], p[:, 2], c[:, 0], c[:, 1]]))
    inx = np.ascontiguousarray(
        np.stack([p[:, 0], p[:, 1], p[:, 2],
                  np.ones(n, np.float32)])).astype(BF16)
    return {"in5": in5, "inx": inx, **stats}


def build_kernel(ntiles=T, num_cores=NCORES, reps=1):
    nc = bacc.Bacc("TRN2", target_bir_lowering=False, debug=False,
                   num_devices=num_cores)
    NP = ntiles * F
    f32 = mybir.dt.float32
    f32r = mybir.dt.float32r
    bf16 = mybir.dt.float16

    in5 = nc.declare_dram_parameter("in5", [5, NP], f32r, isOutput=False)
    inx = nc.declare_dram_parameter("inx", [4, NP], bf16, isOutput=False)
    decls = {}
    for nm, shp, dt_ in (("stP0", [5, 72], f32r), ("bias40", [72, 1], f32),
                         ("stHXa", [16, 128], bf16),
                         ("stHZ", [8, 96], bf16),
                         ("stMAINa", [128, 96], bf16),
                         ("stMAINb", [128, 96], bf16),
                         ("stRED0", [96, 8], bf16),
                         ("stRED1", [96, 8], bf16)):
        decls[nm] = nc.declare_dram_parameter(nm, shp, dt_, isOutput=False)
    out3 = nc.declare_dram_parameter("out3", [3, NP], f32, isOutput=True)

    P2 = 2 * F
    npairs, tail = ntiles // 2, ntiles % 2

    with tile.TileContext(nc) as tc:
        with ExitStack() as ctx:
            stp = ctx.enter_context(tc.tile_pool(name="stats", bufs=1))
            sP0 = stp.tile([5, 72], f32r, tag="sP0")
            sB40 = stp.tile([72, 1], f32, tag="sB40")
            sHXa_t = stp.tile([48, 128], bf16, tag="sHXa")
            sHXa = sHXa_t[32:48, :]
            sHZ_t = stp.tile([72, 96], bf16, tag="sHZ")
            sHZ = sHZ_t[64:72, :]
            sMa = stp.tile([128, 96], bf16, tag="sMa")
            sMb = stp.tile([128, 96], bf16, tag="sMb")
            sRED0 = stp.tile([96, 8], bf16, tag="sRED0")
            sRED1 = stp.tile([96, 8], bf16, tag="sRED1")
            for t_, nm in ((sP0[:], "stP0"), (sB40[:], "bias40"),
                           (sHXa, "stHXa"), (sHZ, "stHZ"),
                           (sMa[:], "stMAINa"), (sMb[:], "stMAINb"),
                           (sRED0[:], "stRED0"), (sRED1[:], "stRED1")):
                nc.sync.dma_start(t_, decls[nm].ap())

            sb_in = ctx.enter_context(tc.tile_pool(name="sb_in", bufs=4))
            sb_mid = ctx.enter_context(tc.tile_pool(name="sb_mid", bufs=3))
            sb_rep = ctx.enter_context(tc.tile_pool(name="sb_rep", bufs=3))
            sb_w = ctx.enter_context(tc.tile_pool(name="sb_w", bufs=2))
            ps_args = ctx.enter_context(
                tc.tile_pool(name="ps_args", bufs=2, space="PSUM"))
            ps_hx = ctx.enter_context(
                tc.tile_pool(name="ps_hx", bufs=1, space="PSUM"))
            ps_cf = ctx.enter_context(
                tc.tile_pool(name="ps_cf", bufs=2, space="PSUM"))
            ps_out = ctx.enter_context(
                tc.tile_pool(name="ps_out", bufs=2, space="PSUM"))

            def mm(out, lhsT, rhs, start=True, stop=True):
                nc.tensor.matmul(out, lhsT, rhs, start=start, stop=stop)

            def pair_body(cols, c0):
                """Emit one pair (or tail) body: `cols` pixels starting at
                full-row column c0."""
                IN5 = sb_in.tile([5, P2], f32r, tag="in5p", name="IN5")
                nc.gpsimd.dma_start(IN5[:, 0:cols],
                                    in5.ap()[:, c0:c0 + cols])
                X96 = sb_in.tile([96, P2], bf16, tag="x96p", name="X96")
                nc.gpsimd.dma_start(
                    X96[:, 0:cols],
                    inx.ap()[:, c0:c0 + cols].partition_broadcast(24))

                hats = sb_mid.tile([72, P2], bf16, tag="hats", name="hats")
                for h in range(0, cols, F):
                    argsP = ps_args.tile([72, F], f32, tag="args",
                                         name="argsP")
                    mm(argsP[:], sP0[:], IN5[:, h:h + F])
                    tabs = sb_mid.tile([72, F], bf16, tag="tabs", name="tabs")
                    nc.scalar.activation(tabs[:], argsP[:],
                                         mybir.ActivationFunctionType.Abs,
                                         bias=sB40[:], scale=1.0)
                    nc.scalar.activation(hats[:, h:h + F], tabs[:],
                                         mybir.ActivationFunctionType.Relu,
                                         bias=1.0, scale=-1.0)

                # hat replications: stride-0 SBUF->SBUF DMAs
                HYa = sb_rep.tile([128, P2], bf16, tag="hya", name="HYa")
                HYb = sb_rep.tile([128, P2], bf16, tag="hyb", name="HYb")
                nc.sync.dma_start(
                    HYa[:, 0:cols],
                    hats[0:8, 0:cols].unsqueeze(1).broadcast_to(
                        (8, 16, cols)))
                nc.sync.dma_start(
                    HYb[:, 0:cols],
                    hats[8:16, 0:cols].unsqueeze(1).broadcast_to(
                        (8, 16, cols)))
                OUTS = sb_in.tile([8, P2], f32, tag="outs", name="OUTS")
                # stationary-major emission: both F-halves back-to-back per
                # stationary so the PE re-uses/overlaps each weight load
                hs = list(range(0, cols, F))
                HZ96s, HZXs, CFs = {}, {}, {}
                HXP = ps_hx.tile([128, P2], f32, tag="hx", name="HXP")
                for h in hs:
                    mm(HXP[:, h:h + F], sHXa, hats[32:48, h:h + F])
                HXS = sb_w.tile([128, P2], bf16, tag="hxs", name="HXS")
                nc.scalar.copy(HXS[:, 0:cols], HXP[:, 0:cols])
                for h in hs:
                    HZ96s[h] = ps_out.tile([96, F], f32, tag="o3",
                                           name="HZ96")
                    mm(HZ96s[h][:], sHZ, hats[64:72, h:h + F])
                Was = sb_w.tile([128, P2], bf16, tag="wa", name="Was")
                nc.vector.tensor_tensor(out=Was[:, 0:cols],
                                        in0=HYa[:, 0:cols],
                                        in1=HXS[:, 0:cols],
                                        op=mybir.AluOpType.mult)
                Wbs = sb_w.tile([128, P2], bf16, tag="wb", name="Wbs")
                nc.vector.tensor_tensor(out=Wbs[:, 0:cols],
                                        in0=HYb[:, 0:cols],
                                        in1=HXS[:, 0:cols],
                                        op=mybir.AluOpType.mult)
                for h in hs:
                    HZXs[h] = sb_w.tile([96, F], bf16, tag="hzx",
                                        name="HZX")
                    nc.vector.tensor_tensor(out=HZXs[h][:],
                                            in0=HZ96s[h][:],
                                            in1=X96[:, h:h + F],
                                            op=mybir.AluOpType.mult)
                for h in hs:
                    CFs[h] = ps_cf.tile([96, F], f32, tag="cf", name="CF")
                    mm(CFs[h][:], sMa[:], Was[:, h:h + F],
                       start=True, stop=False)
                    mm(CFs[h][:], sMb[:], Wbs[:, h:h + F],
                       start=False, stop=True)
                for h in hs:
                    yield (CFs[h], HZXs[h], OUTS, h, (h + F >= cols),
                           c0, cols)

            obank = [None]

            def backend(st):
                CF, HZX, OUTS, h, last, c0, cols = st
                M2 = sb_w.tile([96, F], bf16, tag="m2", name="M2")
                nc.vector.tensor_tensor(out=M2[:], in0=CF[:],
                                        in1=HZX[:],
                                        op=mybir.AluOpType.mult)
                q = h // F
                first = (q == 0)
                if first:
                    obank[0] = ps_out.tile([8, F], f32, tag="o3",
                                           name="OUT3")
                OUT3 = obank[0]
                nq = cols // F
                mm(OUT3[:], (sRED0, sRED1)[q][:], M2[:],
                   start=first, stop=(q == nq - 1))
                if last:
                    nc.scalar.copy(OUTS[0:3 * nq, 0:F], OUT3[0:3 * nq, :])
                    do = bass.AP(
                        tensor=out3.ap().tensor, offset=c0,
                        ap=mybir.VecI64Pair([[F, nq], [NP, 3], [1, F]]))
                    nc.sync.dma_start(do, OUTS[0:3 * nq, 0:F])

            for _rep in range(reps):
                pending = None
                chunks = [(P2, pi * P2) for pi in range(npairs)]
                if tail:
                    chunks.append((F, npairs * P2))
                for cw, cc in chunks:
                    for st in pair_body(cw, cc):
                        if pending is not None:
                            backend(pending)
                        pending = st
                if pending is not None:
                    backend(pending)

    nc.compile()
    return nc


def kernel(pixels: np.ndarray, coords: np.ndarray, grid: np.ndarray) -> np.ndarray:
    assert pixels.shape == (B, H, W, 3) and coords.shape == (B, H, W, 2)
    p = np.asarray(pixels, np.float32).reshape(-1, 3)
    c = np.asarray(coords, np.float32).reshape(-1, 2)
    r = np.ascontiguousarray(p[:, 0]); g = np.ascontiguousarray(p[:, 1])
    b = np.ascontiguousarray(p[:, 2])
    cx = np.ascontiguousarray(c[:, 0]); cy = np.ascontiguousarray(c[:, 1])
    ones = np.ones(NPC, np.float32)

    stats = _cast_stationaries(_make_stationaries(np.asarray(grid, np.float32)))
    in_maps = []
    for cid in range(NCORES):
        s = slice(cid * NPC, (cid + 1) * NPC)
        in5 = np.ascontiguousarray(np.stack([r[s], g[s], b[s], cx[s], cy[s]]))
        inx = np.ascontiguousarray(
            np.stack([r[s], g[s], b[s], ones])).astype(BF16)
        in_maps.append({"in5": in5, "inx": inx, **stats})

    if "nc" not in _CACHE:
        _CACHE["nc"] = build_kernel()
    nc = _CACHE["nc"]
    res = run_bass_kernel_spmd(nc, in_maps, list(range(NCORES)))
    out = np.concatenate([res.results[cid]["out3"].T for cid in range(NCORES)], 0)
    return np.ascontiguousarray(out.reshape(B, H, W, 3).astype(np.float32))

